# revision 12
# baseline (speedup 1.0000x reference)
"""DGT layer Trainium2 kernel (8-core data-parallel over batch).

Math per block (nodes: n=256, edges: n=128; D=128, H=8, dk=16), per batch b:
  Q = h @ Wq;  K = h @ Wk;  V = h @ Wv
  s[i,j,c] = 0.25 * (Q[i,h(c)]. K[j,h(c)]) + e_att[i,j,c]
  p = softmax_j(s);  att[i,c] = sum_j p * (V[j,c] + e_val[i,j,c])
  h1 = BN(att @ Wl + bl + h);  out = BN(h1 + relu(h1@Wf1+bf1)@Wf2+bf2)
BN is training-mode batchnorm over the global (B*n) axis -> cross-core
AllReduce of per-core sum/sumsq.

Kernel strategy (per core, natural [j, c] layout):
  exp factorization: exp(s) = exp(e_att) * exp(0.25*QK)
  E = exp(e_att)            (ACT, fp32->bf16, streamed)
  w = V + e_val             (PE: identity-matmul accumulate, float32r, PSUM)
  U = E * w                 (DVE, bf16 out)
  den/num reductions over j: per-i matmuls with lhsT = EQ_i [j, 8 heads]
    -> [8, c]-shaped "junk block" outputs (only row h(c) of column c matters),
    packed 16 i's per PSUM bank via tile_position column strips + free offsets.
  ratio = num/den (junk entries are harmless positives), bounced through DRAM
  to extract the per-channel diagonal into attT [c, i] for the tail.
  Tail runs transposed ([feature, token]) so BN stats are free-dim reductions
  fused into scalar_tensor_tensor / activation accumulators.
"""

import sys

sys.path.insert(0, "/opt/trn_rl_repo")

import numpy as np

import concourse.bass as bass
import concourse.mybir as mybir
import concourse.tile as tile
from concourse import bacc
from concourse.bass_utils import run_bass_kernel_spmd
from concourse.masks import make_identity

F32 = mybir.dt.float32
F32R = mybir.dt.float32r
BF16 = mybir.dt.bfloat16
AF = mybir.ActivationFunctionType
ALU = mybir.AluOpType

B, N, M, D, H = 8, 256, 128, 128, 8
CORES = list(range(8))

_CACHE = {}


def _col(pool, name, dram_vec, nc, dt=F32):
    """Load a length-128 DRAM vector as a per-partition [128, 1] column."""
    t = pool.tile([128, 1], dt, name=name)
    nc.sync.dma_start(t, dram_vec)
    return t


def _attention(nc, tc, pools, n, ea_d, ev_d, h_d, w_d, scratch, ident_f, ident_r):
    """Build attention for one block. Returns (attT bf16 [128, n], hT f32 [128, n])."""
    sb1, sb_big, sbw, sbs, ps_small, ps_w, ps_acc = pools
    jtc = n // 128  # j partition tiles
    Wq_d, Wk_d, Wv_d = w_d

    # ---- weights ----
    wq = sbw.tile([128, 128], F32, name=f"wq{n}")
    wk = sbw.tile([128, 128], F32, name=f"wk{n}")
    wv = sbw.tile([128, 128], F32, name=f"wv{n}")
    nc.sync.dma_start(wq, Wq_d)
    nc.sync.dma_start(wk, Wk_d)
    nc.sync.dma_start(wv, Wv_d)

    # ---- h, hT ----
    h_sb = sbw.tile([128, n], F32, name=f"h_sb{n}")
    for it in range(jtc):
        nc.sync.dma_start(h_sb[:, it * 128 : (it + 1) * 128], h_d[it * 128 : (it + 1) * 128, :])
    hT = sbw.tile([128, n], F32, name=f"hT{n}")
    for it in range(jtc):
        tp = ps_small.tile([128, 128], F32, name=f"tp{n}", tag="pss")
        nc.tensor.transpose(tp, h_sb[:, it * 128 : (it + 1) * 128], ident_f)
        nc.scalar.copy(out=hT[:, it * 128 : (it + 1) * 128], in_=tp)

    # ---- QT, KT in [k'=16, (h, i)] layout (head in free dim so K=16 slices
    # start at partition 0) ----
    hT_bf = sbw.tile([128, n], BF16, name=f"hTb{n}")
    nc.vector.tensor_copy(hT_bf, hT)
    wq_bf = sbw.tile([128, 128], BF16, name=f"wqb{n}")
    wk_bf = sbw.tile([128, 128], BF16, name=f"wkb{n}")
    nc.vector.tensor_copy(wq_bf, wq)
    nc.vector.tensor_copy(wk_bf, wk)
    qt = sbw.tile([16, H * n], BF16, name=f"qt{n}")
    kt = sbw.tile([16, H * n], BF16, name=f"kt{n}")
    for w_sb, dst in ((wq_bf, qt), (wk_bf, kt)):
        for h in range(H):
            qp = ps_small.tile([16, n], F32, name=f"qp{n}", tag="pss")
            nc.tensor.matmul(qp, w_sb[:, h * 16 : (h + 1) * 16], hT_bf, start=True, stop=True)
            nc.scalar.copy(out=dst[:, h * n : (h + 1) * n], in_=qp)

    # ---- V (f32r, [j, c] per jt side by side) ----
    v_sb = sbw.tile([128, jtc, 128], F32R, name=f"v_sb{n}")
    for jt in range(jtc):
        vp = ps_small.tile([128, 128], F32, name=f"vp{n}", tag="pss")
        nc.tensor.matmul(vp, hT[:, jt * 128 : (jt + 1) * 128], wv, start=True, stop=True)
        nc.vector.tensor_copy(v_sb[:, jt, :], vp)

    # ---- EQ = exp(0.25 * QK): per jt a [j=128, H*n] bf16 buffer, col = h*n + i ----
    eqs = []
    for jt in range(jtc):
        eq = sbw.tile([128, H * n], BF16, name=f"eq{n}_{jt}")
        for h in range(H):
            qkp = ps_small.tile([128, n], F32, name=f"qkp{n}", tag="pss")
            nc.tensor.matmul(
                qkp,
                kt[:, h * n + jt * 128 : h * n + (jt + 1) * 128],
                qt[:, h * n : (h + 1) * n],
                start=True,
                stop=True,
            )
            nc.scalar.activation(
                out=eq[:, h * n : (h + 1) * n], in_=qkp, func=AF.Exp, scale=0.25
            )
        eqs.append(eq)

    # ---- main loop over i in blocks of 16 ----
    for i0 in range(0, n, 16):
        # batched loads: 8 i's per DMA
        batches = []
        for bi in range(2):
            ib = i0 + 8 * bi
            ea8 = sb_big.tile([128, 8, jtc, 128], F32, name=f"ea8_{n}", tag=f"ea8_{n}")
            ev8 = sb_big.tile([128, 8, jtc, 128], F32R, name=f"ev8_{n}", tag=f"ev8_{n}")
            src = bass.AP(
                tensor=ea_d.tensor,
                offset=ib * n * 128,
                ap=[[128, 128], [n * 128, 8], [128 * 128, jtc], [1, 128]],
            )
            nc.sync.dma_start(ea8, src)
            srcv = bass.AP(
                tensor=ev_d.tensor,
                offset=ib * n * 128,
                ap=[[128, 128], [n * 128, 8], [128 * 128, jtc], [1, 128]],
            )
            nc.gpsimd.dma_start(ev8, srcv)
            batches.append((ea8, ev8))

        num_ps = ps_acc.tile([128, 512], F32, name=f"num{n}", tag="accn")
        den_ps = ps_acc.tile([128, 512], F32, name=f"den{n}", tag="accd")
        for g in range(4):  # groups of 4 i
            ea8, ev8 = batches[g // 2]
            gl = (g % 2) * 4  # i offset within the 8i batch
            e_gs, u_gs = [], []
            for jt in range(jtc):
                ea_sl = ea8[:, gl : gl + 4, jt, :]
                ev_sl = ev8[:, gl : gl + 4, jt, :]

                w_ps = ps_w.tile([128, 512], F32, name=f"w_ps{n}", tag="wps")
                nc.tensor.matmul(w_ps.rearrange("p (g c) -> p g c", g=4), ident_r, ev_sl, start=True, stop=False)
                v_rep = bass.AP(
                    tensor=v_sb.tensor,
                    offset=v_sb.offset + jt * 128,
                    ap=[v_sb.ap[0], [0, 4], [1, 128]],
                )
                nc.tensor.matmul(w_ps.rearrange("p (g c) -> p g c", g=4), ident_r, v_rep, start=False, stop=True)

                e_g = sb1.tile([128, 4, 128], BF16, name=f"e_g{n}", tag="e_g")
                nc.scalar.activation(out=e_g, in_=ea_sl, func=AF.Exp)
                u_g = sb1.tile([128, 4, 128], BF16, name=f"u_g{n}", tag="u_g")
                nc.vector.tensor_mul(u_g, e_g, w_ps.rearrange("p (g c) -> p g c", g=4))
                e_gs.append(e_g)
                u_gs.append(u_g)

            for li in range(4):
                loc = g * 4 + li  # 0..15 within block
                i = i0 + loc
                s, q = loc % 4, loc // 4
                for acc, rhss in ((num_ps, u_gs), (den_ps, e_gs)):
                    for jt in range(jtc):
                        eq = eqs[jt]
                        lhsT = bass.AP(
                            tensor=eq.tensor, offset=eq.offset + i, ap=[eq.ap[0], [n, 8]]
                        )
                        nc.tensor.matmul(
                            acc[32 * s : 32 * s + 8, q * 128 : (q + 1) * 128],
                            lhsT,
                            rhss[jt][:, li, :],
                            start=(jt == 0),
                            stop=(jt == jtc - 1),
                            tile_position=(0, 32 * s),
                        )

        rden = sb1.tile([128, 512], F32, name=f"rden{n}", tag="rden")
        nc.vector.reciprocal(rden, den_ps)
        ratio = sb1.tile([128, 512], BF16, name=f"ratio{n}", tag="ratio")
        nc.vector.tensor_mul(ratio, num_ps, rden)
        # scratch[i, h, c] = ratio[(32s+h), (q*128+c)] with i = i0 + q*4 + s
        for s in range(4):
            dst = bass.AP(
                tensor=scratch.tensor,
                offset=scratch.offset + (i0 + s) * 1024,
                ap=[[128, 8], [4 * 1024, 4], [1, 128]],
            )
            nc.sync.dma_start(dst, ratio[32 * s : 32 * s + 8, :].rearrange("p (q c) -> p q c", q=4))

    # ---- read back diagonal: attT[c, i] ----
    attT = sbw.tile([128, n], BF16, name=f"attT{n}")
    for h in range(H):
        src = bass.AP(
            tensor=scratch.tensor,
            offset=scratch.offset + h * 128 + h * 16,
            ap=[[1, 16], [1024, n]],
        )
        nc.sync.dma_start(attT[16 * h : 16 * (h + 1), :], src)
    return attT, hT


def _bn_stats_ar(nc, tc, dram_pool, sb, s_col, q_col, tag):
    """AllReduce [128,2] (sum, sumsq) across cores; returns global [128,2] tile."""
    cin = dram_pool.tile([128, 2], F32, name=f"cin_{tag}")
    cout = dram_pool.tile([128, 2], F32, name=f"cout_{tag}")
    nc.sync.dma_start(cin[:, 0:1], s_col)
    nc.sync.dma_start(cin[:, 1:2], q_col)
    nc.gpsimd.collective_compute(
        "AllReduce",
        ALU.add,
        replica_groups=[CORES],
        ins=[cin.opt()],
        outs=[cout.opt()],
    )
    g = sb.tile([128, 2], F32, name=f"gs_{tag}")
    nc.sync.dma_start(g, cout)
    return g


def _bn_coeffs(nc, sb, gstats, count, g_col, be_col, tag):
    """a = g/sqrt(var+eps), b = be - mu*a  from global (sum, sumsq)."""
    mu = sb.tile([128, 1], F32, name=f"mu_{tag}")
    nc.scalar.mul(out=mu, in_=gstats[:, 0:1], mul=1.0 / count)
    musq = sb.tile([128, 1], F32, name=f"musq_{tag}")
    nc.scalar.square(out=musq, in_=mu)
    var = sb.tile([128, 1], F32, name=f"var_{tag}")
    nc.vector.scalar_tensor_tensor(
        out=var, in0=gstats[:, 1:2], scalar=1.0 / count, in1=musq,
        op0=ALU.mult, op1=ALU.subtract,
    )
    eps = sb.tile([128, 1], F32, name=f"eps_{tag}")
    nc.vector.memset(eps, 1e-5)
    sd = sb.tile([128, 1], F32, name=f"sd_{tag}")
    nc.scalar.activation(out=sd, in_=var, func=AF.Sqrt, bias=eps)
    rs = sb.tile([128, 1], F32, name=f"rs_{tag}")
    nc.vector.reciprocal(rs, sd)
    a = sb.tile([128, 1], F32, name=f"a_{tag}")
    nc.vector.tensor_mul(a, rs, g_col)
    nma = sb.tile([128, 1], F32, name=f"nma_{tag}")
    nc.vector.scalar_tensor_tensor(
        out=nma, in0=mu, scalar=-1.0, in1=a, op0=ALU.mult, op1=ALU.mult
    )
    b = sb.tile([128, 1], F32, name=f"b_{tag}")
    nc.vector.tensor_add(b, nma, be_col)
    return a, b


def _build():
    nc = bacc.Bacc(
        "TRN2",
        target_bir_lowering=False,
        debug=False,
        enable_asserts=False,
        num_devices=8,
    )

    def din(name, shape):
        return nc.dram_tensor(name, shape, F32, kind="ExternalInput").ap()

    h_n = din("h_n", [N, D])
    h_e = din("h_e", [M, D])
    ea_n = din("e_att", [N, N, D])
    ev_n = din("e_val", [N, N, D])
    ea_e = din("e2e_att", [M, M, D])
    ev_e = din("e2e_val", [M, M, D])
    w = {}
    for s in ("n", "e"):
        for nm, shp in (
            ("Wq", [D, D]), ("Wk", [D, D]), ("Wv", [D, D]), ("Wl", [D, D]),
            ("bl", [D]), ("g1", [D]), ("be1", [D]),
            ("Wf1", [D, 2 * D]), ("bf1", [2 * D]),
            ("Wf2", [2 * D, D]), ("bf2", [D]), ("g2", [D]), ("be2", [D]),
        ):
            w[f"{nm}_{s}"] = din(f"{nm}_{s}", shp)
    out_n = nc.dram_tensor("out_n", [N, D], F32, kind="ExternalOutput").ap()
    out_e = nc.dram_tensor("out_e", [M, D], F32, kind="ExternalOutput").ap()
    dbg = {}
    if _CACHE.get("debug"):
        for nm, shp in (("att_n", [128, N]), ("att_e", [128, M]), ("h1_n", [128, N])):
            dbg[nm] = nc.dram_tensor(f"dbg_{nm}", shp, F32, kind="ExternalOutput").ap()

    with tile.TileContext(nc) as tc:
        import contextlib

        with contextlib.ExitStack() as ctx:
            sb1 = ctx.enter_context(tc.tile_pool(name="sb1", bufs=6))
            sb_big = ctx.enter_context(tc.tile_pool(name="sb_big", bufs=3))
            sbw = ctx.enter_context(tc.tile_pool(name="sbw", bufs=1))
            sbs = ctx.enter_context(tc.tile_pool(name="sbs", bufs=2))
            ps_small = ctx.enter_context(tc.tile_pool(name="ps_small", bufs=2, space="PSUM"))
            ps_w = ctx.enter_context(tc.tile_pool(name="ps_w", bufs=2, space="PSUM"))
            ps_acc = ctx.enter_context(tc.tile_pool(name="ps_acc", bufs=2, space="PSUM"))
            dram = ctx.enter_context(tc.tile_pool(name="dram", bufs=1, space="DRAM"))
            pools = (sb1, sb_big, sbw, sbs, ps_small, ps_w, ps_acc)

            ident_f = sbw.tile([128, 128], F32, name="ident_f")
            make_identity(nc, ident_f)
            ident_r = sbw.tile([128, 128], F32R, name="ident_r")
            nc.vector.tensor_copy(ident_r, ident_f)

            scratch_e = dram.tile([M, H, 128], BF16, name="scr_e")
            scratch_n = dram.tile([N, H, 128], BF16, name="scr_n")

            results = {}
            for s, n, ea, ev, h_d, outp in (
                ("e", M, ea_e, ev_e, h_e, out_e),
                ("n", N, ea_n, ev_n, h_n, out_n),
            ):
                scratch = scratch_e if s == "e" else scratch_n
                attT, hT = _attention(
                    nc, tc, pools, n,
                    ea, ev, h_d,
                    (w[f"Wq_{s}"], w[f"Wk_{s}"], w[f"Wv_{s}"]),
                    scratch, ident_f, ident_r,
                )
                if dbg:
                    attf = sbw.tile([128, n], F32, name=f"attf_{s}")
                    nc.vector.tensor_copy(attf, attT)
                    nc.sync.dma_start(dbg[f"att_{s}"], attf)

                # ---- h1T = Wl.T@attT + bl + hT ; BN1 stats ----
                wl = sbw.tile([128, 128], F32, name=f"wl_{s}")
                nc.sync.dma_start(wl, w[f"Wl_{s}"])
                wl_bf = sbw.tile([128, 128], BF16, name=f"wlb_{s}")
                nc.vector.tensor_copy(wl_bf, wl)
                h1p = ps_small.tile([128, n], F32, name=f"h1p_{s}", tag="pss")
                nc.tensor.matmul(h1p, wl_bf, attT, start=True, stop=True)

                bl_c = _col(sbw, f"bl_{s}", w[f"bl_{s}"], nc)
                h1T = sbw.tile([128, n], F32, name=f"h1T_{s}")
                s1 = sbw.tile([128, 1], F32, name=f"s1_{s}")
                nc.vector.scalar_tensor_tensor(
                    out=h1T, in0=h1p, scalar=bl_c, in1=hT,
                    op0=ALU.add, op1=ALU.add, accum_out=s1,
                )
                sqt = sbs.tile([128, n], F32, name=f"sqt_{s}", tag="sqt")
                q1 = sbw.tile([128, 1], F32, name=f"q1_{s}")
                nc.scalar.activation(out=sqt, in_=h1T, func=AF.Square, accum_out=q1)
                if dbg and s == "n":
                    nc.sync.dma_start(dbg["h1_n"], h1T)
                g1_ar = _bn_stats_ar(nc, tc, dram, sbw, s1, q1, f"1{s}")
                results[s] = (n, attT, hT, h1T, g1_ar, outp)

            for s in ("e", "n"):
                n, attT, hT, h1T, g1_ar, outp = results[s]
                g1_c = _col(sbw, f"g1c_{s}", w[f"g1_{s}"], nc)
                be1_c = _col(sbw, f"be1c_{s}", w[f"be1_{s}"], nc)
                a1, b1 = _bn_coeffs(nc, sbw, g1_ar, B * n, g1_c, be1_c, f"1{s}")
                h1n = sbw.tile([128, n], F32, name=f"h1n_{s}")
                nc.scalar.activation(out=h1n, in_=h1T, func=AF.Identity, bias=b1, scale=a1)
                h1nb = sbw.tile([128, n], BF16, name=f"h1nb_{s}")
                nc.vector.tensor_copy(h1nb, h1n)

                # ---- FFN ----
                wf1 = sbw.tile([128, 256], F32, name=f"wf1_{s}")
                nc.sync.dma_start(wf1, w[f"Wf1_{s}"])
                wf1b = sbw.tile([128, 256], BF16, name=f"wf1b_{s}")
                nc.vector.tensor_copy(wf1b, wf1)
                wf2 = sbw.tile([128, 2, 128], F32, name=f"wf2_{s}")
                src = bass.AP(
                    tensor=w[f"Wf2_{s}"].tensor, offset=0,
                    ap=[[128, 128], [128 * 128, 2], [1, 128]],
                )
                nc.sync.dma_start(wf2, src)
                wf2b = sbw.tile([128, 2, 128], BF16, name=f"wf2b_{s}")
                nc.vector.tensor_copy(wf2b, wf2)
                bf1_cols = sbw.tile([128, 2], F32, name=f"bf1c_{s}")
                nc.sync.dma_start(bf1_cols, bass.AP(tensor=w[f"bf1_{s}"].tensor, offset=0, ap=[[1, 128], [128, 2]]))

                x2p = ps_small.tile([128, n], F32, name=f"x2p_{s}", tag="pss")
                for ft in range(2):
                    ffp = ps_small.tile([128, n], F32, name=f"ffp_{s}", tag="pss")
                    nc.tensor.matmul(ffp, wf1b[:, ft * 128 : (ft + 1) * 128], h1nb, start=True, stop=True)
                    ff1 = sbs.tile([128, n], BF16, name=f"ff1_{s}", tag="ff1")
                    nc.scalar.activation(out=ff1, in_=ffp, func=AF.Relu, bias=bf1_cols[:, ft : ft + 1])
                    nc.tensor.matmul(x2p, wf2b[:, ft, :], ff1, start=(ft == 0), stop=(ft == 1))

                bf2_c = _col(sbw, f"bf2c_{s}", w[f"bf2_{s}"], nc)
                x2T = sbw.tile([128, n], F32, name=f"x2T_{s}")
                s2 = sbw.tile([128, 1], F32, name=f"s2_{s}")
                nc.vector.scalar_tensor_tensor(
                    out=x2T, in0=x2p, scalar=bf2_c, in1=h1n,
                    op0=ALU.add, op1=ALU.add, accum_out=s2,
                )
                sq2 = sbs.tile([128, n], F32, name=f"sq2_{s}", tag="sqt")
                q2 = sbw.tile([128, 1], F32, name=f"q2_{s}")
                nc.scalar.activation(out=sq2, in_=x2T, func=AF.Square, accum_out=q2)
                g2_ar = _bn_stats_ar(nc, tc, dram, sbw, s2, q2, f"2{s}")
                results[s] = (n, x2T, g2_ar, outp)

            for s in ("e", "n"):
                n, x2T, g2_ar, outp = results[s]
                g2_c = _col(sbw, f"g2c_{s}", w[f"g2_{s}"], nc)
                be2_c = _col(sbw, f"be2c_{s}", w[f"be2_{s}"], nc)
                a2, b2 = _bn_coeffs(nc, sbw, g2_ar, B * n, g2_c, be2_c, f"2{s}")
                oT = sbw.tile([128, n], F32, name=f"oT_{s}")
                nc.scalar.activation(out=oT, in_=x2T, func=AF.Identity, bias=b2, scale=a2)
                for it in range(n // 128):
                    tp = ps_small.tile([128, 128], F32, name=f"otp_{s}", tag="pss")
                    nc.tensor.transpose(tp, oT[:, it * 128 : (it + 1) * 128], ident_f)
                    ob = sbs.tile([128, 128], F32, name=f"ob_{s}", tag="ob")
                    nc.scalar.copy(out=ob, in_=tp)
                    nc.sync.dma_start(outp[it * 128 : (it + 1) * 128, :], ob)

    nc.compile()
    return nc


def _get_nc():
    if "nc" not in _CACHE:
        _CACHE["nc"] = _build()
    return _CACHE["nc"]


def kernel(**inputs):
    nc = _get_nc()
    wnames = [
        f"{nm}_{s}"
        for s in ("n", "e")
        for nm in ("Wq", "Wk", "Wv", "Wl", "bl", "g1", "be1", "Wf1", "bf1", "Wf2", "bf2", "g2", "be2")
    ]
    in_maps = []
    for b in range(8):
        m = {
            "h_n": np.ascontiguousarray(inputs["h_n"][b * N : (b + 1) * N]),
            "h_e": np.ascontiguousarray(inputs["h_e"][b * M : (b + 1) * M]),
            "e_att": np.ascontiguousarray(inputs["e_att"][b]),
            "e_val": np.ascontiguousarray(inputs["e_val"][b]),
            "e2e_att": np.ascontiguousarray(inputs["e2e_att"][b]),
            "e2e_val": np.ascontiguousarray(inputs["e2e_val"][b]),
        }
        for k in wnames:
            m[k] = np.ascontiguousarray(inputs[k])
        in_maps.append(m)
    res = run_bass_kernel_spmd(nc, in_maps, CORES).results
    out_n = np.concatenate([res[b]["out_n"] for b in range(8)], axis=0)
    out_e = np.concatenate([res[b]["out_e"] for b in range(8)], axis=0)
    return out_n, out_e
